# revision 4
# baseline (speedup 1.0000x reference)
"""ContinuousTimeRNN Trainium2 kernel, v10 (v8 + split relu).

W-stationary transposed-delta design (see v4) with a 100-step For_i body:
4 windows of 25 steps, ping-pong hist, two static x-buffers prefetched a
half-body ahead, spread y-flush, DVE epilogue copies.
"""

import sys

sys.path.insert(0, "/opt/trn_rl_repo")

import numpy as np

ALPHA = 0.1
T, N, H, DIN, DOUT, INIT = 1000, 512, 512, 2, 2, 2
NCORES = 8
NS = N // NCORES          # 64 batch rows per core
NK = H // 128             # 4 H-chunks
WIN = 25                  # h-history window (steps)
NW = 4                    # windows per For_i body
BODY = NW * WIN           # 100 steps per body
XB = 2 * WIN              # steps covered by one x-buffer
QW = WIN * NS // 4        # 400 columns per y-flush quarter


def _build_nc(t_total=T, reps=1):
    import concourse.mybir as mybir
    from concourse import bacc
    from concourse.tile import TileContext
    from concourse.bass import ds

    fp32 = mybir.dt.float32
    fp16 = mybir.dt.float16
    fp8 = mybir.dt.float8e4
    AF = mybir.ActivationFunctionType
    ALU = mybir.AluOpType

    nc = bacc.Bacc("TRN2", target_bir_lowering=False, debug=False,
                   num_devices=NCORES)

    assert t_total % BODY == 0

    # -------- DRAM I/O (per core) --------
    wrec_d = nc.dram_tensor("wrec", [NK * NK, 128, 128], fp8, kind="ExternalInput").ap()
    win3_d = nc.dram_tensor("win3", [DIN + 1, H], fp16, kind="ExternalInput").ap()
    ident_d = nc.dram_tensor("ident9", [128, 128], fp16, kind="ExternalInput").ap()
    wout_d = nc.dram_tensor("wout", [NK, 128, DOUT], fp16, kind="ExternalInput").ap()
    fcw3_d = nc.dram_tensor("fcw3", [INIT + 1, H], fp32, kind="ExternalInput").ap()
    init3_d = nc.dram_tensor("init3", [INIT + 1, NS], fp32, kind="ExternalInput").ap()
    # padded by one body so the last prefetch stays in range
    xt_d = nc.dram_tensor("xt", [DIN + 1, (t_total + BODY) * NS], fp16,
                          kind="ExternalInput").ap()
    # padded by one window at the front (first-body flush scratch)
    y_d = nc.dram_tensor("y", [DOUT, (t_total + WIN) * NS], fp32,
                         kind="ExternalOutput").ap()

    with TileContext(nc) as tc:
        with (
            tc.tile_pool(name="wpool", bufs=1) as wpool,
            tc.tile_pool(name="hpool", bufs=1) as hpool,
            tc.tile_pool(name="apool", bufs=3) as apool,
            tc.tile_pool(name="ypool", bufs=2) as ypool,
            tc.tile_pool(name="hps", bufs=3, space="PSUM") as hps,
            tc.tile_pool(name="yps", bufs=2, space="PSUM") as yps,
        ):
            # -------- persistent SBUF --------
            wrec_sb = wpool.tile([128, NK * NK, 128], fp8)    # 6.4*W_rec chunk (k,m)
            win3_sb = wpool.tile([DIN + 1, H], fp16)          # 0.1*[W_in; bias]
            ident_sb = wpool.tile([128, 128], fp16)           # 921.6*I
            wout_sb = wpool.tile([128, NK, DOUT], fp16)       # W_out chunks
            fcw3_sb = wpool.tile([INIT + 1, H], fp32)         # [fc_w.T; fc_b]
            init3_sb = wpool.tile([INIT + 1, NS], fp32)       # [initdir.T; ones]
            xba = wpool.tile([DIN + 1, XB * NS], fp16)        # x cols, windows 0-1
            xbb = wpool.tile([DIN + 1, XB * NS], fp16)        # x cols, windows 2-3
            hist_a = hpool.tile([128, NK, WIN * NS], fp16)
            hist_b = hpool.tile([128, NK, WIN * NS], fp16)
            hist = [hist_a, hist_b]
            # first-body y quarters read hist[1] before it's fully written
            # (results land in the y padding); zero both hist tiles once
            nc.vector.memset(hist_a[:], 0.0)
            nc.vector.memset(hist_b[:], 0.0)

            for i in range(NK * NK):
                nc.sync.dma_start(out=wrec_sb[:, i, :], in_=wrec_d[i])
            for k in range(NK):
                nc.sync.dma_start(out=wout_sb[:, k, :], in_=wout_d[k])
            nc.sync.dma_start(out=win3_sb[:], in_=win3_d)
            nc.sync.dma_start(out=ident_sb[:], in_=ident_d)
            nc.sync.dma_start(out=fcw3_sb[:], in_=fcw3_d)
            nc.sync.dma_start(out=init3_sb[:], in_=init3_d)
            nc.sync.dma_start(out=xba[:], in_=xt_d[:, 0: XB * NS])

            # -------- h0 = fc(initdir) -> hist[1] slot WIN-1 (fp16) --------
            ph0 = hps.tile([128, NK * NS], fp32)
            for m in range(NK):
                nc.tensor.matmul(ph0[:, m * NS:(m + 1) * NS],
                                 fcw3_sb[:, m * 128:(m + 1) * 128],
                                 init3_sb[:], start=True, stop=True)
            nc.vector.tensor_copy(
                hist[1][:, :, (WIN - 1) * NS: WIN * NS],
                ph0[:].rearrange("p (k n) -> p k n", k=NK))

            # -------- time loop: NW windows per body --------
            with tc.For_i(0, reps, 1) as _rep, tc.For_i(0, t_total, BODY) as iv:
                ps_prev = None
                for w in range(NW):
                    hc, hp = hist[w % 2], hist[1 - (w % 2)]
                    pair = w // 2
                    xbuf = [xba, xbb][pair % 2]
                    if w % 2 == 0:
                        # prefetch the next window-pair's x buffer
                        nxt = [xba, xbb][(pair + 1) % 2]
                        nc.sync.dma_start(
                            out=nxt[:],
                            in_=xt_d[:, ds(iv * NS + (pair + 1) * XB * NS,
                                           XB * NS)])
                    for s in range(WIN):
                        prev_slot = (hp[:, :, (WIN - 1) * NS: WIN * NS] if s == 0
                                     else hc[:, :, (s - 1) * NS: s * NS])
                        tt = apool.tile([128, NK * NS], fp16, tag="tt")
                        a = apool.tile([128, NK * NS], fp16, tag="a")
                        if ps_prev is None:
                            # body boundary: tanh from SBUF hist slot
                            ttv = tt[:].rearrange("p (k n) -> p k n", k=NK)
                            nc.scalar.activation(ttv, prev_slot, AF.Tanh)
                        else:
                            # psum carries 64*h
                            nc.scalar.activation(tt[:], ps_prev[:], AF.Tanh,
                                                 scale=1.0 / 64.0)
                            nc.scalar.activation(
                                prev_slot,
                                ps_prev[:].rearrange("p (k n) -> p k n",
                                                     k=NK),
                                AF.Copy, scale=1.0 / 64.0)
                        # a = relu(tt) fp16 in halves (wrec k0,1 start early)
                        HB = NK * NS // 2
                        nc.vector.tensor_scalar_max(a[:, 0:HB], tt[:, 0:HB],
                                                    0.0)
                        nc.vector.tensor_scalar_max(a[:, HB:2 * HB],
                                                    tt[:, HB:2 * HB], 0.0)

                        # PE block: one accumulation group for the whole bank.
                        xcol = ((w % 2) * WIN + s) * NS
                        psb = hps.tile([128, 512], fp32, tag="psb")
                        ps = psb[:, 0:NK * NS]
                        for m in range(NK):
                            nc.tensor.matmul(ps[:, m * NS:(m + 1) * NS],
                                             win3_sb[:, m * 128:(m + 1) * 128],
                                             xbuf[:, xcol:xcol + NS],
                                             start=(m == 0), stop=False)
                        for k in range(NK):
                            for m in range(NK):
                                nc.tensor.matmul(ps[:, m * NS:(m + 1) * NS],
                                                 wrec_sb[:, k * NK + m, :],
                                                 a[:, k * NS:(k + 1) * NS],
                                                 start=False, stop=False)
                        nc.tensor.matmul(
                            ps.rearrange("p (k n) -> p k n", k=NK),
                            ident_sb[:], prev_slot,
                            start=False, stop=True)
                        ps_prev = ps

                        # spread y flush of the previous window into the
                        # tanh/relu idle: quarter q at step s=2q+2 (writes
                        # land in the padded y at offset (window-1)*WIN*NS)
                        if s in (2, 4, 6, 8):
                            q = s // 2 - 1
                            yp = yps.tile([DOUT, QW], fp32)
                            for k in range(NK):
                                nc.tensor.matmul(yp[:], wout_sb[:, k, :],
                                                 hp[:, k, q * QW:(q + 1) * QW],
                                                 start=(k == 0),
                                                 stop=(k == NK - 1))
                            ysb = ypool.tile([DOUT, QW], fp32, tag="ysb")
                            nc.scalar.copy(out=ysb[:], in_=yp[:])
                            nc.sync.dma_start(
                                out=y_d[:, ds(iv * NS + w * WIN * NS + q * QW,
                                              QW)],
                                in_=ysb[:])

                    if w == NW - 1:
                        # body epilogue: last h of the body -> hc slot WIN-1
                        # (psum carries 1024*h)
                        nc.vector.tensor_scalar(
                            hc[:, :, (WIN - 1) * NS: WIN * NS],
                            ps_prev[:].rearrange("p (k n) -> p k n", k=NK),
                            1.0 / 64.0, None, ALU.mult)
                        ps_prev = None

            # final window: hist[1] -> y cols [t_total*NS, (t_total+WIN)*NS)
            for q in range(4):
                yp = yps.tile([DOUT, QW], fp32)
                for k in range(NK):
                    nc.tensor.matmul(yp[:], wout_sb[:, k, :],
                                     hist[1][:, k, q * QW:(q + 1) * QW],
                                     start=(k == 0), stop=(k == NK - 1))
                ysb = ypool.tile([DOUT, QW], fp32, tag="ysb")
                nc.vector.tensor_copy(ysb[:], yp[:])
                nc.sync.dma_start(
                    out=y_d[:, t_total * NS + q * QW: t_total * NS + (q + 1) * QW],
                    in_=ysb[:])

    nc.compile()
    return nc


_NC_CACHE = {}


def _get_nc():
    if "nc" not in _NC_CACHE:
        _NC_CACHE["nc"] = _build_nc()
    return _NC_CACHE["nc"]


def _prep_in_maps(initdir, velocities, fc_w, fc_b, W_in, W_rec, W_out, bias):
    initdir = np.asarray(initdir, np.float32)
    velocities = np.asarray(velocities, np.float32)
    fc_w = np.asarray(fc_w, np.float32)
    fc_b = np.asarray(fc_b, np.float32)
    W_in = np.asarray(W_in, np.float32)
    W_rec = np.asarray(W_rec, np.float32)
    W_out = np.asarray(W_out, np.float32)
    bias = np.asarray(bias, np.float32)

    import ml_dtypes
    f8 = np.dtype(ml_dtypes.float8_e4m3fn)
    wt = (64.0 * ALPHA * W_rec).astype(f8)                       # (512, 512)
    wrec = np.empty((NK * NK, 128, 128), f8)
    for k in range(NK):
        for m in range(NK):
            wrec[k * NK + m] = wt[k * 128:(k + 1) * 128, m * 128:(m + 1) * 128]
    win3 = (64.0 * ALPHA *
            np.concatenate([W_in, bias[None, :]], axis=0)).astype(np.float16)
    ident9 = (57.6 * np.eye(128, dtype=np.float32)).astype(np.float16)
    wout = W_out.reshape(NK, 128, DOUT).astype(np.float16)
    fcw3 = np.concatenate([fc_w.T, fc_b[None, :]], axis=0).astype(np.float32)

    in_maps = []
    for c in range(NCORES):
        sl = slice(c * NS, (c + 1) * NS)
        init3 = np.concatenate([initdir[sl].T,
                                np.ones((1, NS), np.float32)], axis=0)
        xs = velocities[:, sl, :]                                # (T, NS, 2)
        xt = np.zeros((DIN + 1, (T + BODY) * NS), np.float16)
        xt[:DIN, 0:T * NS] = (
            xs.transpose(2, 0, 1).reshape(DIN, T * NS).astype(np.float16))
        xt[DIN] = 1.0
        in_maps.append({
            "wrec": wrec,
            "win3": win3,
            "ident9": ident9,
            "wout": wout,
            "fcw3": fcw3,
            "init3": np.ascontiguousarray(init3),
            "xt": xt,
        })
    return in_maps


def kernel(initdir, velocities, fc_w, fc_b, W_in, W_rec, W_out, bias):
    from concourse.bass_utils import run_bass_kernel_spmd

    in_maps = _prep_in_maps(initdir, velocities, fc_w, fc_b, W_in, W_rec,
                            W_out, bias)
    nc = _get_nc()
    res = run_bass_kernel_spmd(nc, in_maps, list(range(NCORES)))

    out = np.empty((T, N, DOUT), np.float32)
    for c in range(NCORES):
        yt = res.results[c]["y"][:, WIN * NS:]                   # (2, T*NS)
        out[:, c * NS:(c + 1) * NS, :] = (
            yt.reshape(DOUT, T, NS).transpose(1, 2, 0))
    return out
